# revision 11
# baseline (speedup 1.0000x reference)
"""Causal self-attention (B=4, T=2048, C=1024, H=16) on 8 trn2 NeuronCores.

Sharding: core c -> batch b = c//2, heads h0 = (c%2)*8 .. h0+8 (tensor
parallel over heads: c_attn columns / c_proj rows split). Each core computes a
partial projection output [T, C] in bf16; the host sums the two partials per
batch and adds b_proj.

Device-side dataflow (the kernel is exp-stream-bound: the Scalar engine's
~150us of exp is the critical path, so everything is organized around
starting it early and never starving it):
  - host passes x[b] pre-transposed twice: xt [C, T] bf16 (v path) and
    xt8 [C, T] fp8e4 (q/k path); wq/wk are fp8e4 scaled by 32
  - q/k projections: fp8 DoubleRow matmuls folding two 128-row contraction
    tiles per pass (2x PE throughput); PSUM evacuated on DVE tensor_scalar
    (scale 1/32 + bq bias for q; bk is DROPPED entirely - a per-query-constant
    logit shift cancels in softmax). Softmax tolerates the ~2% fp8 q/k noise;
    v and the output path stay bf16.
  - loads are 12 large DMAs on the sync/gpsimd queues only (descriptor
    issuance on the scalar queue would delay the first exp)
  - stage B proper is only the 0:512 column block of q/k per head pair plus
    its pass-0 S tiles and v tiles 0..3: first exp fires ~10us in. The
    remaining q/k column blocks and v tiles run as fillers inside the
    attention loop, emitted on demand (pull_s emits the q/k blocks a pass
    needs before its S tiles).
  - S^T tiles: TWO concurrent row-tiled bf16 matmuls (contraction 64 each,
    array rows 0:63 / 64:127) into one psS tile [128,2,512]
  - P~ = exp(S^T/8) on ScalarE, one instruction per psS tile (both heads);
    diagonal 128x128 blocks masked with an upper-triangular mask on DVE
  - attention runs in four 512-wide column passes; the output projection for
    pass c-1 overlaps the attention of pass c; the final four projection
    tiles are pipelined against the last pass's normalizes with copies split
    across Scalar and DVE
  - yT_aug [MV, 1024] += vaug_tile^T . P~ accumulated in PSUM over k-tiles
  - normalize: DVE reciprocal of denom row, gpsimd partition-broadcast,
    DVE multiply into yT (bf16)
"""

import numpy as np

P = 128


def _bf16_np():
    import ml_dtypes
    return ml_dtypes.bfloat16


def _f8_np():
    import ml_dtypes
    return ml_dtypes.float8_e4m3


WS = 32.0  # host-side scale on wq/wk before fp8 quantization


def build_program(T=2048, C=1024, HC=8, D=64, num_devices=8, trn="TRN2"):
    import concourse.mybir as mybir
    import concourse.tile as tile
    from concourse import bacc
    from concourse.masks import make_upper_triangular

    W = 512          # matmul moving-dim chunk (PSUM bank)
    KC = C // P      # contraction tiles over C (8)
    KP = KC // 2     # fp8 DoubleRow contraction pair-tiles (4)
    CO = HC * D      # this core's qkv channel block (512)
    NP = CO // P     # head pairs (4)
    TT = T // P      # k tiles (16)
    MV = 66          # PV stationary cols: 64 v-dims + ones + 1 pad
    dt32 = mybir.dt.float32
    bf16 = mybir.dt.bfloat16
    fp8 = mybir.dt.float8e4
    ActF = mybir.ActivationFunctionType
    Alu = mybir.AluOpType
    DR = mybir.MatmulPerfMode.DoubleRow
    scale = 1.0 / float(np.sqrt(D))

    nc = bacc.Bacc(trn, target_bir_lowering=False, debug=False,
                   enable_asserts=False, num_devices=num_devices)

    # all bulk inputs arrive pre-arranged partition-major on the host so
    # every load is one contiguous run per partition (descriptor generation
    # for strided gathers costs ~7us per DMA on the issuing queue)
    xt_d = nc.dram_tensor("xt", [P, KC, T], bf16, kind="ExternalInput")
    xt8_d = nc.dram_tensor("xt8", [P, KP, 2, T], fp8, kind="ExternalInput")
    wq_d = nc.dram_tensor("wq", [P, KP, 2, CO], fp8, kind="ExternalInput")
    wk_d = nc.dram_tensor("wk", [P, KP, 2, CO], fp8, kind="ExternalInput")
    wv_d = nc.dram_tensor("wv", [P, KC, CO], bf16, kind="ExternalInput")
    bq_d = nc.dram_tensor("bq", [P, NP], dt32, kind="ExternalInput")
    bvb_d = nc.dram_tensor("bvb", [P, CO], dt32, kind="ExternalInput")
    wp_d = nc.dram_tensor("wp", [P, NP, C], bf16, kind="ExternalInput")
    out_d = nc.dram_tensor("out", [T, C], bf16, kind="ExternalOutput")
    lsc_d = nc.dram_tensor("lsc", [HC, T], dt32)
    lsc2_d = nc.dram_tensor("lsc2", [HC, T], dt32)

    with tile.TileContext(nc) as tc:
        with tc.tile_pool(name="const", bufs=1) as cpool, \
             tc.tile_pool(name="pers", bufs=1) as pers:
            tri2 = cpool.tile([P, 2, P], bf16)
            make_upper_triangular(nc, tri2[:, 0, :], val=1.0, diag=True)
            make_upper_triangular(nc, tri2[:, 1, :], val=1.0, diag=True)
            bq_sb = cpool.tile([P, NP], dt32)
            bvb_sb = cpool.tile([P, CO], dt32)
            wpsb = cpool.tile([P, NP, C], bf16)

            qT = pers.tile([P, NP, T], bf16, tag="qT")
            kT = pers.tile([P, NP, T], bf16, tag="kT")
            vaug = pers.tile([P, TT, HC, MV], bf16, tag="vaug")
            yT = pers.tile([P, NP, T], bf16, tag="yT")
            nc.vector.memset(vaug[:, :, :, D:D + 1], 1.0)

            # big consolidated input tiles (one DMA each)
            xts = pers.tile([P, KC, T], bf16, tag="xts")
            x8t = pers.tile([P, KP, 2, T], fp8, tag="x8t")
            wqt = pers.tile([P, KP, 2, CO], fp8, tag="wqt")
            wkt = pers.tile([P, KP, 2, CO], fp8, tag="wkt")
            wvt = pers.tile([P, KC, CO], bf16, tag="wvt")

            from contextlib import ExitStack
            outer = ExitStack()
            ptpool = outer.enter_context(tc.tile_pool(name="ptp", bufs=20))
            psS = outer.enter_context(
                tc.tile_pool(name="psS", bufs=2, space="PSUM"))

            # ---- loads: sync/gpsimd only, critical first ------------------
            # first exp needs only x8 column block 0 + wq/wk head pair 0:
            # load those six small chunks first, everything else after
            di = [0]

            def dma(dst, src):
                [nc.sync, nc.gpsimd][di[0] % 2].dma_start(dst, src)
                di[0] += 1

            for kk in range(KP):
                dma(x8t[:, kk, :, 0:W], xt8_d.ap()[:, kk, :, 0:W])
            dma(wqt[:, :, :, 0:P], wq_d.ap()[:, :, :, 0:P])
            dma(wkt[:, :, :, 0:P], wk_d.ap()[:, :, :, 0:P])
            nc.sync.dma_start(bq_sb[:], bq_d.ap())
            for m in range(1, NP):
                dma(wqt[:, :, :, m * P:(m + 1) * P],
                    wq_d.ap()[:, :, :, m * P:(m + 1) * P])
                dma(wkt[:, :, :, m * P:(m + 1) * P],
                    wk_d.ap()[:, :, :, m * P:(m + 1) * P])
            for tq in range(1, NP):
                for kk in range(KP):
                    dma(x8t[:, kk, :, tq * W:(tq + 1) * W],
                        xt8_d.ap()[:, kk, :, tq * W:(tq + 1) * W])
            nc.gpsimd.dma_start(xts[:, 0:KC // 2, :],
                                xt_d.ap()[:, 0:KC // 2, :])
            nc.sync.dma_start(xts[:, KC // 2:, :], xt_d.ap()[:, KC // 2:, :])
            nc.gpsimd.dma_start(wvt[:], wv_d.ap())
            nc.sync.dma_start(bvb_sb[:], bvb_d.ap())
            nc.gpsimd.dma_start(wpsb[:], wp_d.ap())
            bvb_v = bvb_sb[:].rearrange("p (h d) -> p h d", d=D)

            def emit_s(m, j, plo, phi):
                """Paired S^T + exp for heads (2m, 2m+1), k-tile j,
                columns [max(jb,plo), phi). Returns the pt pair tile."""
                jb = j * P
                qlo = max(jb, plo)
                w = phi - qlo
                pt = ptpool.tile([P, 2, W], bf16, tag="pt")
                sps = psS.tile([P, 2, W], dt32, tag="s")
                nc.tensor.matmul(
                    sps[:, 0, 0:w],
                    kT[0:D, m, jb:jb + P],
                    qT[0:D, m, qlo:phi],
                    start=True, stop=True, skip_group_check=True)
                nc.tensor.matmul(
                    sps[:, 1, 0:w],
                    kT[D:P, m, jb:jb + P],
                    qT[D:P, m, qlo:phi],
                    start=True, stop=True, skip_group_check=True)
                nc.scalar.activation(
                    pt[:, :, 0:w], sps[:, :, 0:w], ActF.Exp, scale=scale)
                if jb >= plo:  # diagonal block lives in this pass
                    # gpsimd, not DVE: a busy DVE queue here would delay the
                    # diagonal PV at the PE queue head and stall the S stream
                    nc.gpsimd.tensor_mul(pt[:, :, 0:P], pt[:, :, 0:P],
                                         tri2[:])
                return pt

            def qk_group(wt, dst, m, tq, bias, pool):
                ps = pool.tile([P, W], dt32, tag=pool_tag[id(pool)],
                               name=f"qk_{m}_{tq}")
                for kk in range(KP):
                    nc.tensor.matmul(
                        ps[:],
                        wt[:, kk, :, m * P:(m + 1) * P],
                        x8t[:, kk, :, tq * W:(tq + 1) * W],
                        start=(kk == 0), stop=(kk == KP - 1),
                        perf_mode=DR, skip_group_check=True)
                if bias is not None:
                    nc.vector.tensor_scalar(
                        out=dst[:, m, tq * W:(tq + 1) * W], in0=ps[:],
                        scalar1=1.0 / WS, scalar2=bias,
                        op0=Alu.mult, op1=Alu.add)
                else:
                    nc.vector.tensor_scalar(
                        out=dst[:, m, tq * W:(tq + 1) * W], in0=ps[:],
                        scalar1=1.0 / WS, scalar2=None, op0=Alu.mult)

            def v_evac(tt, ps):
                nc.vector.scalar_tensor_tensor(
                    out=vaug[:, tt, :, 0:D],
                    in0=ps[:].rearrange("p (h d) -> p h d", d=D),
                    scalar=1.0, in1=bvb_v,
                    op0=Alu.mult, op1=Alu.add)

            # ---- stage B: q/k column block 0 + pass-0 S + v tiles 0..3 ---
            pass0_pts = [[] for _ in range(NP)]
            pool_tag = {}
            with nc.named_scope("qkv"), \
                 tc.tile_pool(name="psB", bufs=4, space="PSUM") as psB:
                pool_tag[id(psB)] = "psB"
                for m in range(NP):
                    qk_group(wqt, qT, m, 0, bq_sb[:, m:m + 1], psB)
                    qk_group(wkt, kT, m, 0, None, psB)
                    for j in range(4):
                        pass0_pts[m].append((j, emit_s(m, j, 0, W)))
                for tt in range(4):
                    ps = psB.tile([P, CO], dt32, tag="psB", name="vps")
                    for kc in range(KC):
                        nc.tensor.matmul(
                            ps[:],
                            xts[:, kc, tt * P:(tt + 1) * P],
                            wvt[:, kc, :],
                            start=(kc == 0), stop=(kc == KC - 1))
                    v_evac(tt, ps)

            # ------- attention + projection -------------------------------
            # four 512-wide column sub-passes; pass c consumes k-tiles
            # j <= 4c+3. q/k column blocks 1..3, v tiles 4..15 and the
            # previous pass's projection tiles all run as fillers trickled
            # between S chunks so no engine bursts ever starve the exp
            # stream.
            with nc.named_scope("attn"), \
                 tc.tile_pool(name="nrm", bufs=4) as nrmpool, \
                 tc.tile_pool(name="ost", bufs=2) as opool, \
                 tc.tile_pool(name="psY", bufs=3, space="PSUM") as psY, \
                 tc.tile_pool(name="psO", bufs=1, space="PSUM") as psO:
                pool_tag[id(psO)] = "o"

                def make_qk_piece(m, tq):
                    def q_piece():
                        qk_group(wqt, qT, m, tq, bq_sb[:, m:m + 1], psO)

                    def k_piece():
                        qk_group(wkt, kT, m, tq, None, psO)

                    return q_piece, k_piece

                def make_v_pieces(tt):
                    st = {}

                    def p1():
                        st["ps"] = psO.tile([P, CO], dt32, tag="o",
                                            name="vps")
                        for kc in range(KC // 2):
                            nc.tensor.matmul(
                                st["ps"][:],
                                xts[:, kc, tt * P:(tt + 1) * P],
                                wvt[:, kc, :],
                                start=(kc == 0), stop=False)

                    def p2():
                        for kc in range(KC // 2, KC):
                            nc.tensor.matmul(
                                st["ps"][:],
                                xts[:, kc, tt * P:(tt + 1) * P],
                                wvt[:, kc, :],
                                start=False, stop=(kc == KC - 1))
                        v_evac(tt, st["ps"])

                    return [p1, p2]

                def make_proj_pieces(tt):
                    st = {}

                    def mk(nn, half):
                        def piece():
                            if half == 0:
                                st[nn] = psO.tile([P, W], dt32, tag="o",
                                                  name=f"po{nn}")
                                if nn == 0:
                                    st["ot"] = opool.tile([P, C], bf16,
                                                          tag="ot",
                                                          name="ot")
                                kts = (0, 1)
                            else:
                                kts = (2, 3)
                            for kt in kts:
                                nc.tensor.matmul(
                                    st[nn][:],
                                    yT[:, kt, tt * P:(tt + 1) * P],
                                    wpsb[:, kt, nn * W:(nn + 1) * W],
                                    start=(kt == 0), stop=(kt == NP - 1),
                                    skip_group_check=True)
                            if half == 1:
                                nc.vector.tensor_copy(
                                    st["ot"][:, nn * W:(nn + 1) * W],
                                    st[nn][:])
                                if nn == 1:
                                    [nc.sync, nc.gpsimd][tt % 2].dma_start(
                                        out_d.ap()[tt * P:(tt + 1) * P, :],
                                        st["ot"][:])
                        return piece

                    return [mk(0, 0), mk(0, 1), mk(1, 0), mk(1, 1)]

                # filler entries: (kind, key, fn); kind 'v' keyed by tt,
                # kind 'qk' keyed by (m, tq), kind 'p' for proj pieces
                fillers = []
                qk_pending = {}
                for tq in range(1, NP):
                    for m in range(NP):
                        qp, kp = make_qk_piece(m, tq)
                        fillers.append(("qk", (m, tq, "q"), qp))
                        fillers.append(("qk", (m, tq, "k"), kp))
                        qk_pending[(m, tq, "q")] = qp
                        qk_pending[(m, tq, "k")] = kp
                    for tt in range(4 * tq, 4 * tq + 4):
                        fillers.extend(
                            ("v", tt, f) for f in make_v_pieces(tt))

                def run_entry(e):
                    """Returns True if real work was emitted."""
                    kind, key, fn = e
                    if kind == "qk":
                        if key in qk_pending:
                            del qk_pending[key]
                            fn()
                            return True
                        return False
                    fn()
                    return True

                def pump(n=1):
                    done = 0
                    while done < n and fillers:
                        if run_entry(fillers.pop(0)):
                            done += 1

                def ensure_qk(m, tq):
                    for kq in ("q", "k"):
                        key = (m, tq, kq)
                        if key in qk_pending:
                            fn = qk_pending.pop(key)
                            fn()

                def pump_v_upto(tt):
                    while fillers and any(
                            k == "v" and key <= tt
                            for k, key, _ in fillers):
                        run_entry(fillers.pop(0))

                def emit_pv(h, j, pt, yt, plo, phi, jmax):
                    jb = j * P
                    qlo = max(jb, plo)
                    nc.tensor.matmul(
                        yt[:, qlo - plo:phi - plo],
                        vaug[:, j, h, :],
                        pt[:, h % 2, 0:phi - qlo],
                        start=(j == 0), stop=(j == jmax),
                        skip_group_check=True)

                # finish is a 3-stage pipeline across head-pairs so no DVE op
                # ever waits at the head of the queue on an in-flight DMA:
                #   front: evacuate yt PSUM + kick the denom-row fold DMAs
                #   mid (a pair later): reciprocal + kick the broadcast DMAs
                #   back (another pair later): normalize-multiply into yT
                fin_q1, fin_q2 = [], []

                def finish_front(h, c, yt, plo, phi):
                    ys = nrmpool.tile([D + 1, W], dt32, tag="ys")
                    nc.vector.tensor_copy(ys[:], yt[0:D + 1, :])
                    nc.sync.dma_start(
                        lsc_d.ap()[h, plo:phi].rearrange("(o t) -> o t", o=1),
                        ys[D:D + 1, :])
                    dn = nrmpool.tile([P, W // P], dt32, tag="dn")
                    nc.gpsimd.dma_start(
                        dn[:],
                        lsc_d.ap()[h, plo:phi].rearrange("(p c) -> p c", p=P))
                    fin_q1.append((h, ys, dn, plo, phi))

                def finish_mid(st):
                    h, ys, dn, plo, phi = st
                    nc.vector.reciprocal(dn[:], dn[:])
                    nc.gpsimd.dma_start(
                        lsc2_d.ap()[h, plo:phi].rearrange("(p c) -> p c", p=P),
                        dn[:])
                    bc = nrmpool.tile([D, W], dt32, tag="bc")
                    nc.sync.dma_start(
                        bc[:],
                        lsc2_d.ap()[h, plo:phi].rearrange(
                            "(o t) -> o t", o=1).broadcast_to([D, W]))
                    fin_q2.append((h, ys, bc, plo, phi))

                def finish_back(st):
                    h, ys, bc, plo, phi = st
                    r0 = (h % 2) * D
                    nc.vector.tensor_mul(
                        yT[r0:r0 + D, h // 2, plo:phi], ys[0:D, :], bc[:])

                def finish_step():
                    while len(fin_q1) > 2:
                        finish_mid(fin_q1.pop(0))
                    while len(fin_q2) > 2:
                        finish_back(fin_q2.pop(0))

                def finish_flush():
                    while fin_q1:
                        finish_mid(fin_q1.pop(0))
                    while fin_q2:
                        finish_back(fin_q2.pop(0))

                def finish_fast(h, yt, plo, phi):
                    """DMA-free normalize (gpsimd broadcast + fast DVE
                    reciprocal) — low latency, for the last column pass."""
                    drow = nrmpool.tile([1, W], dt32, tag="drow")
                    nc.vector.tensor_copy(drow[:], yt[D:D + 1, :])
                    ys = nrmpool.tile([D + 1, W], dt32, tag="ys")
                    nc.vector.tensor_copy(ys[0:D, :], yt[0:D, :])
                    bc = nrmpool.tile([D, W], dt32, tag="bc")
                    nc.gpsimd.partition_broadcast(bc[:], drow[:], channels=D)
                    rec = nrmpool.tile([D, W], dt32, tag="bc", name="rec")
                    nc.vector.reciprocal_approx_fast(out=rec[:], in_=bc[:])
                    r0 = (h % 2) * D
                    nc.vector.tensor_mul(
                        yT[r0:r0 + D, h // 2, plo:phi], ys[0:D, :], rec[:])

                def emit_proj_mms(tt, pos, k0, k1):
                    for kt in range(k0, k1):
                        for nn in range(2):
                            nc.tensor.matmul(
                                pos[nn][:],
                                yT[:, kt, tt * P:(tt + 1) * P],
                                wpsb[:, kt, nn * W:(nn + 1) * W],
                                start=(kt == 0), stop=(kt == NP - 1),
                                skip_group_check=True)

                def emit_proj_done(tt, pos, split=False, q=None):
                    ot = opool.tile([P, C], bf16, tag="ot")
                    if split:
                        # kernel tail: Scalar is idle, split the two PSUM
                        # evacuation copies across Scalar and DVE
                        nc.scalar.copy(ot[:, 0:W], pos[0][:])
                        nc.vector.tensor_copy(ot[:, W:2 * W], pos[1][:])
                    else:
                        for nn in range(2):
                            nc.vector.tensor_copy(
                                ot[:, nn * W:(nn + 1) * W], pos[nn][:])
                    if q is None:
                        q = [nc.sync, nc.gpsimd][tt % 2]
                    q.dma_start(out_d.ap()[tt * P:(tt + 1) * P, :], ot[:])

                # Global S-emission cursor kept ~14 chunks ahead of PV
                # consumption; pull_s first emits any q/k column blocks its
                # chunk depends on. Pass 0 S tiles were pre-built in stage B.
                sq = [(c2, m2, j2) for c2 in range(1, 4) for m2 in range(NP)
                      for j2 in range(4 * c2 + 4)]
                sq_i = 0
                pts_q = {(0, m2): list(pass0_pts[m2]) for m2 in range(NP)}
                state = {"ahead": sum(len(v) for v in pts_q.values())}

                def pull_s():
                    c2, m2, j2 = sq[sq_i]
                    ensure_qk(m2, c2)          # qT columns of this pass
                    for tqk in range(1, c2 + 1):
                        ensure_qk(m2, tqk)     # kT columns up to this pass
                    pt = emit_s(m2, j2, c2 * W, (c2 + 1) * W)
                    pts_q.setdefault((c2, m2), []).append((j2, pt))
                    state["ahead"] += 1

                # proj tiles of pass c-1 interleave with pass c, placed so
                # every yT write they read is already emitted
                proj_hooks = {}
                for c2 in range(1, 4):
                    proj_hooks[c2, 1] = [4 * (c2 - 1)]
                    proj_hooks[c2, 2] = [4 * (c2 - 1) + 1]
                    proj_hooks[c2, 3] = [4 * (c2 - 1) + 2, 4 * (c2 - 1) + 3]

                for c in range(4):
                    plo, phi = c * W, (c + 1) * W
                    jmax = 4 * c + 3
                    # v-tiles this pass reads must be emitted before its
                    # first PV; later fillers keep trickling
                    pump_v_upto(jmax)
                    for m in range(NP):
                        hA, hB = 2 * m, 2 * m + 1
                        ytA = psY.tile([MV, W], dt32, tag="yt", name="ytA")
                        ytB = psY.tile([MV, W], dt32, tag="yt", name="ytB")
                        while (len(pts_q.get((c, m), [])) < jmax + 1
                               and sq_i < len(sq)):
                            pull_s()
                            sq_i += 1
                            pump()
                        for pj, ppt in pts_q.pop((c, m)):
                            emit_pv(hA, pj, ppt, ytA, plo, phi, jmax)
                            emit_pv(hB, pj, ppt, ytB, plo, phi, jmax)
                            state["ahead"] -= 1
                            while state["ahead"] < 16 and sq_i < len(sq):
                                pull_s()
                                sq_i += 1
                                pump()
                        if c < 3 or m < NP - 1:
                            finish_front(hA, c, ytA, plo, phi)
                            finish_front(hB, c, ytB, plo, phi)
                            finish_step()
                            for tt in proj_hooks.get((c, m), []):
                                fillers.extend(
                                    ("p", -1, f)
                                    for f in make_proj_pieces(tt))
                        else:
                            finish_flush()
                            for tt in proj_hooks.get((c, m), []):
                                fillers.extend(
                                    ("p", -1, f)
                                    for f in make_proj_pieces(tt))
                            pump(len(fillers))
                            # tail: sq is exhausted here, psS slots are free
                            # for the last four proj tiles; copies split
                            # across Scalar+DVE, stores spread over queues
                            pop12 = psS.tile([P, 2, W], dt32, tag="s",
                                             name="pop12")
                            pos12 = [pop12[:, 0, :], pop12[:, 1, :]]
                            emit_proj_mms(12, pos12, 0, NP - 1)
                            finish_fast(hA, ytA, plo, phi)
                            finish_fast(hB, ytB, plo, phi)
                            emit_proj_mms(12, pos12, NP - 1, NP)
                            emit_proj_done(12, pos12, split=True,
                                           q=nc.sync)
                            pop13 = psS.tile([P, 2, W], dt32, tag="s",
                                             name="pop13")
                            pos13 = [pop13[:, 0, :], pop13[:, 1, :]]
                            emit_proj_mms(13, pos13, 0, NP)
                            emit_proj_done(13, pos13, split=True,
                                           q=nc.scalar)
                            pop14 = psS.tile([P, 2, W], dt32, tag="s",
                                             name="pop14")
                            pos14 = [pop14[:, 0, :], pop14[:, 1, :]]
                            emit_proj_mms(14, pos14, 0, NP)
                            emit_proj_done(14, pos14, split=True,
                                           q=nc.sync)
                            pop15 = psS.tile([P, 2, W], dt32, tag="s",
                                             name="pop15")
                            pos15 = [pop15[:, 0, :], pop15[:, 1, :]]
                            emit_proj_mms(15, pos15, 0, NP)
                            emit_proj_done(15, pos15, split=True,
                                           q=nc.scalar)
            outer.close()

    nc.compile()
    return nc


def make_core_inputs(x, W_attn, b_attn, W_proj, n_cores=8, HC=8, D=64):
    """Host-side sharding: per-core input dicts."""
    B, T, C = x.shape
    CO = HC * D
    NP = CO // P
    bf = _bf16_np()
    f8 = _f8_np()
    in_maps = []
    for c in range(n_cores):
        b = c // (n_cores // B)
        h0 = (c % (n_cores // B)) * HC
        lo = h0 * D
        bq = b_attn[lo:lo + CO]
        bv = b_attn[2 * C + lo:2 * C + lo + CO]
        xtb = x[b].T  # [C, T]
        T = xtb.shape[1]
        C_ = xtb.shape[0]
        KC = C_ // P
        KP = KC // 2

        def pmaj(a, *groups):
            # [C_, n] with C_=(g0 g1 .. p) -> [P, g0, g1, .., n]
            shp = list(groups) + [P, a.shape[1]]
            nd = len(shp)
            perm = [nd - 2] + list(range(nd - 2)) + [nd - 1]
            return np.ascontiguousarray(a.reshape(shp).transpose(perm))

        in_maps.append({
            "xt": pmaj(xtb.astype(bf), KC),
            "xt8": pmaj(xtb.astype(f8), KP, 2),
            "wq": pmaj((W_attn[:, lo:lo + CO] * WS).astype(f8), KP, 2),
            "wk": pmaj((W_attn[:, C + lo:C + lo + CO] * WS).astype(f8),
                       KP, 2),
            "wv": pmaj(W_attn[:, 2 * C + lo:2 * C + lo + CO].astype(bf),
                       KC),
            "bq": np.ascontiguousarray(bq.reshape(NP, P).T),
            "bvb": np.tile(bv[None, :], (P, 1)),
            "wp": pmaj(W_proj[lo:lo + CO, :].astype(bf), NP),
        })
    return in_maps


_CACHE = {}


def _get_program():
    if "nc" not in _CACHE:
        _CACHE["nc"] = build_program()
    return _CACHE["nc"]


def run_on_cores(x, W_attn, b_attn, W_proj, b_proj, trace=False):
    """Returns (full output [B,T,C], BassKernelResults)."""
    from concourse.bass_utils import run_bass_kernel_spmd

    x = np.asarray(x, np.float32)
    W_attn = np.asarray(W_attn, np.float32)
    b_attn = np.asarray(b_attn, np.float32)
    W_proj = np.asarray(W_proj, np.float32)
    b_proj = np.asarray(b_proj, np.float32)

    nc = _get_program()
    in_maps = make_core_inputs(x, W_attn, b_attn, W_proj)
    res = run_bass_kernel_spmd(nc, in_maps, core_ids=list(range(8)), trace=trace)
    B, T, C = x.shape
    out = np.empty((B, T, C), np.float32)
    for b in range(B):
        out[b] = (res.results[2 * b]["out"].astype(np.float32)
                  + res.results[2 * b + 1]["out"].astype(np.float32)
                  + b_proj[None, :])
    return out, res


def kernel(x, W_attn, b_attn, W_proj, b_proj):
    out, _ = run_on_cores(x, W_attn, b_attn, W_proj, b_proj, trace=False)
    return out


# revision 12
# speedup vs baseline: 1.3533x; 1.3533x over previous
"""Causal self-attention (B=4, T=2048, C=1024, H=16) on 8 trn2 NeuronCores.

Sharding: core c -> batch b = c//2, heads h0 = (c%2)*8 .. h0+8 (tensor
parallel over heads: c_attn columns / c_proj rows split). Each core computes a
partial projection output [T, C] in bf16; the host sums the two partials per
batch and adds b_proj.

Device-side dataflow (the kernel is exp-stream-bound: the Scalar engine's
~150us of exp is the critical path, so everything is organized around
starting it early and never starving it):
  - host passes x[b] pre-transposed twice: xt [C, T] bf16 (v path) and
    xt8 [C, T] fp8e4 (q/k path); wq/wk are fp8e4 scaled by 32
  - q/k projections: fp8 DoubleRow matmuls folding two 128-row contraction
    tiles per pass (2x PE throughput); PSUM evacuated on DVE tensor_scalar
    (scale 1/32 + bq bias for q; bk is DROPPED entirely - a per-query-constant
    logit shift cancels in softmax). Softmax tolerates the ~2% fp8 q/k noise;
    v and the output path stay bf16.
  - loads are 12 large DMAs on the sync/gpsimd queues only (descriptor
    issuance on the scalar queue would delay the first exp)
  - stage B proper is only the 0:512 column block of q/k per head pair plus
    its pass-0 S tiles and v tiles 0..3: first exp fires ~10us in. The
    remaining q/k column blocks and v tiles run as fillers inside the
    attention loop, emitted on demand (pull_s emits the q/k blocks a pass
    needs before its S tiles).
  - S^T tiles: TWO concurrent row-tiled bf16 matmuls (contraction 64 each,
    array rows 0:63 / 64:127) into one psS tile [128,2,512]
  - P~ = exp(S^T/8) on ScalarE, one instruction per psS tile (both heads);
    diagonal 128x128 blocks masked with an upper-triangular mask on DVE
  - attention runs in four 512-wide column passes; the output projection for
    pass c-1 overlaps the attention of pass c; the final four projection
    tiles are pipelined against the last pass's normalizes with copies split
    across Scalar and DVE
  - yT_aug [MV, 1024] += vaug_tile^T . P~ accumulated in PSUM over k-tiles
  - normalize: DVE reciprocal of denom row, gpsimd partition-broadcast,
    DVE multiply into yT (bf16)
"""

import numpy as np

P = 128


def _bf16_np():
    import ml_dtypes
    return ml_dtypes.bfloat16


def _f8_np():
    import ml_dtypes
    return ml_dtypes.float8_e4m3


WS = 32.0  # host-side scale on wq/wk before fp8 quantization


def build_program(T=2048, C=1024, HC=8, D=64, num_devices=8, trn="TRN2"):
    import concourse.mybir as mybir
    import concourse.tile as tile
    from concourse import bacc
    from concourse.masks import make_upper_triangular

    W = 512          # matmul moving-dim chunk (PSUM bank)
    KC = C // P      # contraction tiles over C (8)
    KP = KC // 2     # fp8 DoubleRow contraction pair-tiles (4)
    CO = HC * D      # this core's qkv channel block (512)
    NP = CO // P     # head pairs (4)
    TT = T // P      # k tiles (16)
    MV = 66          # PV stationary cols: 64 v-dims + ones + 1 pad
    dt32 = mybir.dt.float32
    bf16 = mybir.dt.bfloat16
    fp8 = mybir.dt.float8e4
    ActF = mybir.ActivationFunctionType
    Alu = mybir.AluOpType
    DR = mybir.MatmulPerfMode.DoubleRow
    scale = 1.0 / float(np.sqrt(D))

    nc = bacc.Bacc(trn, target_bir_lowering=False, debug=False,
                   enable_asserts=False, num_devices=num_devices)

    # all bulk inputs arrive pre-arranged partition-major on the host so
    # every load is one contiguous run per partition (descriptor generation
    # for strided gathers costs ~7us per DMA on the issuing queue)
    xt_d = nc.dram_tensor("xt", [P, KC, T], bf16, kind="ExternalInput")
    xt8_d = nc.dram_tensor("xt8", [P, KP, 2, T], fp8, kind="ExternalInput")
    wq_d = nc.dram_tensor("wq", [P, KP, 2, CO], fp8, kind="ExternalInput")
    wk_d = nc.dram_tensor("wk", [P, KP, 2, CO], fp8, kind="ExternalInput")
    wv_d = nc.dram_tensor("wv", [P, KC, CO], bf16, kind="ExternalInput")
    bq_d = nc.dram_tensor("bq", [P, NP], dt32, kind="ExternalInput")
    bvb_d = nc.dram_tensor("bvb", [P, CO], dt32, kind="ExternalInput")
    wp_d = nc.dram_tensor("wp", [P, NP, C], bf16, kind="ExternalInput")
    out_d = nc.dram_tensor("out", [T, C], bf16, kind="ExternalOutput")
    lsc_d = nc.dram_tensor("lsc", [HC, T], dt32)
    lsc2_d = nc.dram_tensor("lsc2", [HC, T], dt32)

    with tile.TileContext(nc) as tc:
        with tc.tile_pool(name="const", bufs=1) as cpool, \
             tc.tile_pool(name="pers", bufs=1) as pers:
            tri2 = cpool.tile([P, 2, P], bf16)
            make_upper_triangular(nc, tri2[:, 0, :], val=1.0, diag=True)
            make_upper_triangular(nc, tri2[:, 1, :], val=1.0, diag=True)
            bq_sb = cpool.tile([P, NP], dt32)
            bvb_sb = cpool.tile([P, CO], dt32)
            wpsb = cpool.tile([P, NP, C], bf16)

            qT = pers.tile([P, NP, T], bf16, tag="qT")
            kT = pers.tile([P, NP, T], bf16, tag="kT")
            vaug = pers.tile([P, TT, HC, MV], bf16, tag="vaug")
            yT = pers.tile([P, NP, T], bf16, tag="yT")
            nc.vector.memset(vaug[:, :, :, D:D + 1], 1.0)

            # big consolidated input tiles (one DMA each)
            xts = pers.tile([P, KC, T], bf16, tag="xts")
            x8t = pers.tile([P, KP, 2, T], fp8, tag="x8t")
            wqt = pers.tile([P, KP, 2, CO], fp8, tag="wqt")
            wkt = pers.tile([P, KP, 2, CO], fp8, tag="wkt")
            wvt = pers.tile([P, KC, CO], bf16, tag="wvt")

            from contextlib import ExitStack
            outer = ExitStack()
            ptpool = outer.enter_context(tc.tile_pool(name="ptp", bufs=20))
            psS = outer.enter_context(
                tc.tile_pool(name="psS", bufs=2, space="PSUM"))

            # ---- loads: sync/gpsimd only, critical first ------------------
            # first exp needs only x8 column block 0 + wq/wk head pair 0:
            # load those six small chunks first, everything else after
            di = [0]

            def dma(dst, src):
                [nc.sync, nc.gpsimd][di[0] % 2].dma_start(dst, src)
                di[0] += 1

            for kk in range(KP):
                dma(x8t[:, kk, :, 0:W], xt8_d.ap()[:, kk, :, 0:W])
            dma(wqt[:, :, :, 0:P], wq_d.ap()[:, :, :, 0:P])
            dma(wkt[:, :, :, 0:P], wk_d.ap()[:, :, :, 0:P])
            nc.sync.dma_start(bq_sb[:], bq_d.ap())
            for m in range(1, NP):
                dma(wqt[:, :, :, m * P:(m + 1) * P],
                    wq_d.ap()[:, :, :, m * P:(m + 1) * P])
                dma(wkt[:, :, :, m * P:(m + 1) * P],
                    wk_d.ap()[:, :, :, m * P:(m + 1) * P])
            for tq in range(1, NP):
                for kk in range(KP):
                    dma(x8t[:, kk, :, tq * W:(tq + 1) * W],
                        xt8_d.ap()[:, kk, :, tq * W:(tq + 1) * W])
            nc.gpsimd.dma_start(xts[:, 0:KC // 2, :],
                                xt_d.ap()[:, 0:KC // 2, :])
            nc.sync.dma_start(xts[:, KC // 2:, :], xt_d.ap()[:, KC // 2:, :])
            nc.gpsimd.dma_start(wvt[:], wv_d.ap())
            nc.sync.dma_start(bvb_sb[:], bvb_d.ap())
            nc.gpsimd.dma_start(wpsb[:], wp_d.ap())
            bvb_v = bvb_sb[:].rearrange("p (h d) -> p h d", d=D)

            def emit_s(m, j, plo, phi):
                """Paired S^T + exp for heads (2m, 2m+1), k-tile j,
                columns [max(jb,plo), phi). Returns the pt pair tile."""
                jb = j * P
                qlo = max(jb, plo)
                w = phi - qlo
                pt = ptpool.tile([P, 2, W], bf16, tag="pt")
                sps = psS.tile([P, 2, W], dt32, tag="s")
                nc.tensor.matmul(
                    sps[:, 0, 0:w],
                    kT[0:D, m, jb:jb + P],
                    qT[0:D, m, qlo:phi],
                    start=True, stop=True, skip_group_check=True)
                nc.tensor.matmul(
                    sps[:, 1, 0:w],
                    kT[D:P, m, jb:jb + P],
                    qT[D:P, m, qlo:phi],
                    start=True, stop=True, skip_group_check=True)
                nc.scalar.activation(
                    pt[:, :, 0:w], sps[:, :, 0:w], ActF.Exp, scale=scale)
                if jb >= plo:  # diagonal block lives in this pass
                    nc.vector.tensor_mul(pt[:, :, 0:P], pt[:, :, 0:P],
                                         tri2[:])
                return pt

            def qk_group(wt, dst, m, tq, bias, pool):
                ps = pool.tile([P, W], dt32, tag=pool_tag[id(pool)],
                               name=f"qk_{m}_{tq}")
                for kk in range(KP):
                    nc.tensor.matmul(
                        ps[:],
                        wt[:, kk, :, m * P:(m + 1) * P],
                        x8t[:, kk, :, tq * W:(tq + 1) * W],
                        start=(kk == 0), stop=(kk == KP - 1),
                        perf_mode=DR, skip_group_check=True)
                if bias is not None:
                    nc.vector.tensor_scalar(
                        out=dst[:, m, tq * W:(tq + 1) * W], in0=ps[:],
                        scalar1=1.0 / WS, scalar2=bias,
                        op0=Alu.mult, op1=Alu.add)
                else:
                    nc.vector.tensor_scalar(
                        out=dst[:, m, tq * W:(tq + 1) * W], in0=ps[:],
                        scalar1=1.0 / WS, scalar2=None, op0=Alu.mult)

            def v_evac(tt, ps):
                nc.vector.scalar_tensor_tensor(
                    out=vaug[:, tt, :, 0:D],
                    in0=ps[:].rearrange("p (h d) -> p h d", d=D),
                    scalar=1.0, in1=bvb_v,
                    op0=Alu.mult, op1=Alu.add)

            # ---- stage B: q/k column block 0 + pass-0 S + v tiles 0..3 ---
            pass0_pts = [[] for _ in range(NP)]
            pool_tag = {}
            with nc.named_scope("qkv"), \
                 tc.tile_pool(name="psB", bufs=4, space="PSUM") as psB:
                pool_tag[id(psB)] = "psB"
                for m in range(NP):
                    qk_group(wqt, qT, m, 0, bq_sb[:, m:m + 1], psB)
                    qk_group(wkt, kT, m, 0, None, psB)
                    for j in range(4):
                        pass0_pts[m].append((j, emit_s(m, j, 0, W)))
                for tt in range(4):
                    ps = psB.tile([P, CO], dt32, tag="psB", name="vps")
                    for kc in range(KC):
                        nc.tensor.matmul(
                            ps[:],
                            xts[:, kc, tt * P:(tt + 1) * P],
                            wvt[:, kc, :],
                            start=(kc == 0), stop=(kc == KC - 1))
                    v_evac(tt, ps)

            # ------- attention + projection -------------------------------
            # four 512-wide column sub-passes; pass c consumes k-tiles
            # j <= 4c+3. q/k column blocks 1..3, v tiles 4..15 and the
            # previous pass's projection tiles all run as fillers trickled
            # between S chunks so no engine bursts ever starve the exp
            # stream.
            with nc.named_scope("attn"), \
                 tc.tile_pool(name="nrm", bufs=4) as nrmpool, \
                 tc.tile_pool(name="ost", bufs=2) as opool, \
                 tc.tile_pool(name="psY", bufs=3, space="PSUM") as psY, \
                 tc.tile_pool(name="psO", bufs=1, space="PSUM") as psO:
                pool_tag[id(psO)] = "o"

                def make_qk_piece(m, tq):
                    def q_piece():
                        qk_group(wqt, qT, m, tq, bq_sb[:, m:m + 1], psO)

                    def k_piece():
                        qk_group(wkt, kT, m, tq, None, psO)

                    return q_piece, k_piece

                def make_v_pieces(tt):
                    st = {}

                    def p1():
                        st["ps"] = psO.tile([P, CO], dt32, tag="o",
                                            name="vps")
                        for kc in range(KC // 2):
                            nc.tensor.matmul(
                                st["ps"][:],
                                xts[:, kc, tt * P:(tt + 1) * P],
                                wvt[:, kc, :],
                                start=(kc == 0), stop=False)

                    def p2():
                        for kc in range(KC // 2, KC):
                            nc.tensor.matmul(
                                st["ps"][:],
                                xts[:, kc, tt * P:(tt + 1) * P],
                                wvt[:, kc, :],
                                start=False, stop=(kc == KC - 1))
                        v_evac(tt, st["ps"])

                    return [p1, p2]

                def make_proj_pieces(tt):
                    st = {}

                    def mk(nn, half):
                        def piece():
                            if half == 0:
                                st[nn] = psO.tile([P, W], dt32, tag="o",
                                                  name=f"po{nn}")
                                if nn == 0:
                                    st["ot"] = opool.tile([P, C], bf16,
                                                          tag="ot",
                                                          name="ot")
                                kts = (0, 1)
                            else:
                                kts = (2, 3)
                            for kt in kts:
                                nc.tensor.matmul(
                                    st[nn][:],
                                    yT[:, kt, tt * P:(tt + 1) * P],
                                    wpsb[:, kt, nn * W:(nn + 1) * W],
                                    start=(kt == 0), stop=(kt == NP - 1),
                                    skip_group_check=True)
                            if half == 1:
                                nc.vector.tensor_copy(
                                    st["ot"][:, nn * W:(nn + 1) * W],
                                    st[nn][:])
                                if nn == 1:
                                    [nc.sync, nc.gpsimd][tt % 2].dma_start(
                                        out_d.ap()[tt * P:(tt + 1) * P, :],
                                        st["ot"][:])
                        return piece

                    return [mk(0, 0), mk(0, 1), mk(1, 0), mk(1, 1)]

                # filler entries: (kind, key, fn); kind 'v' keyed by tt,
                # kind 'qk' keyed by (m, tq), kind 'p' for proj pieces
                fillers = []
                qk_pending = {}
                for tq in range(1, NP):
                    for m in range(NP):
                        qp, kp = make_qk_piece(m, tq)
                        fillers.append(("qk", (m, tq, "q"), qp))
                        fillers.append(("qk", (m, tq, "k"), kp))
                        qk_pending[(m, tq, "q")] = qp
                        qk_pending[(m, tq, "k")] = kp
                    for tt in range(4 * tq, 4 * tq + 4):
                        fillers.extend(
                            ("v", tt, f) for f in make_v_pieces(tt))

                def run_entry(e):
                    """Returns True if real work was emitted."""
                    kind, key, fn = e
                    if kind == "qk":
                        if key in qk_pending:
                            del qk_pending[key]
                            fn()
                            return True
                        return False
                    fn()
                    return True

                def pump(n=1):
                    done = 0
                    while done < n and fillers:
                        if run_entry(fillers.pop(0)):
                            done += 1

                def ensure_qk(m, tq):
                    for kq in ("q", "k"):
                        key = (m, tq, kq)
                        if key in qk_pending:
                            fn = qk_pending.pop(key)
                            fn()

                def pump_v_upto(tt):
                    while fillers and any(
                            k == "v" and key <= tt
                            for k, key, _ in fillers):
                        run_entry(fillers.pop(0))

                def emit_pv(h, j, pt, yt, plo, phi, jmax):
                    jb = j * P
                    qlo = max(jb, plo)
                    nc.tensor.matmul(
                        yt[:, qlo - plo:phi - plo],
                        vaug[:, j, h, :],
                        pt[:, h % 2, 0:phi - qlo],
                        start=(j == 0), stop=(j == jmax),
                        skip_group_check=True)

                # finish is a 3-stage pipeline across head-pairs so no DVE op
                # ever waits at the head of the queue on an in-flight DMA:
                #   front: evacuate yt PSUM + kick the denom-row fold DMAs
                #   mid (a pair later): reciprocal + kick the broadcast DMAs
                #   back (another pair later): normalize-multiply into yT
                fin_q1, fin_q2 = [], []

                def finish_front(h, c, yt, plo, phi):
                    ys = nrmpool.tile([D + 1, W], dt32, tag="ys")
                    nc.vector.tensor_copy(ys[:], yt[0:D + 1, :])
                    nc.sync.dma_start(
                        lsc_d.ap()[h, plo:phi].rearrange("(o t) -> o t", o=1),
                        ys[D:D + 1, :])
                    dn = nrmpool.tile([P, W // P], dt32, tag="dn")
                    nc.gpsimd.dma_start(
                        dn[:],
                        lsc_d.ap()[h, plo:phi].rearrange("(p c) -> p c", p=P))
                    fin_q1.append((h, ys, dn, plo, phi))

                def finish_mid(st):
                    h, ys, dn, plo, phi = st
                    nc.vector.reciprocal(dn[:], dn[:])
                    nc.gpsimd.dma_start(
                        lsc2_d.ap()[h, plo:phi].rearrange("(p c) -> p c", p=P),
                        dn[:])
                    bc = nrmpool.tile([D, W], dt32, tag="bc")
                    nc.sync.dma_start(
                        bc[:],
                        lsc2_d.ap()[h, plo:phi].rearrange(
                            "(o t) -> o t", o=1).broadcast_to([D, W]))
                    fin_q2.append((h, ys, bc, plo, phi))

                def finish_back(st):
                    h, ys, bc, plo, phi = st
                    r0 = (h % 2) * D
                    nc.vector.tensor_mul(
                        yT[r0:r0 + D, h // 2, plo:phi], ys[0:D, :], bc[:])

                def finish_step():
                    while len(fin_q1) > 2:
                        finish_mid(fin_q1.pop(0))
                    while len(fin_q2) > 2:
                        finish_back(fin_q2.pop(0))

                def finish_flush():
                    while fin_q1:
                        finish_mid(fin_q1.pop(0))
                    while fin_q2:
                        finish_back(fin_q2.pop(0))

                def finish_fast(h, yt, plo, phi):
                    """DMA-free normalize (gpsimd broadcast + fast DVE
                    reciprocal) — low latency, for the last column pass."""
                    drow = nrmpool.tile([1, W], dt32, tag="drow")
                    nc.vector.tensor_copy(drow[:], yt[D:D + 1, :])
                    ys = nrmpool.tile([D + 1, W], dt32, tag="ys")
                    nc.vector.tensor_copy(ys[0:D, :], yt[0:D, :])
                    bc = nrmpool.tile([D, W], dt32, tag="bc")
                    nc.gpsimd.partition_broadcast(bc[:], drow[:], channels=D)
                    rec = nrmpool.tile([D, W], dt32, tag="bc", name="rec")
                    nc.vector.reciprocal_approx_fast(out=rec[:], in_=bc[:])
                    r0 = (h % 2) * D
                    nc.vector.tensor_mul(
                        yT[r0:r0 + D, h // 2, plo:phi], ys[0:D, :], rec[:])

                def emit_proj_mms(tt, pos, k0, k1):
                    for kt in range(k0, k1):
                        for nn in range(2):
                            nc.tensor.matmul(
                                pos[nn][:],
                                yT[:, kt, tt * P:(tt + 1) * P],
                                wpsb[:, kt, nn * W:(nn + 1) * W],
                                start=(kt == 0), stop=(kt == NP - 1),
                                skip_group_check=True)

                def emit_proj_done(tt, pos, split=False, q=None):
                    ot = opool.tile([P, C], bf16, tag="ot")
                    if split:
                        # kernel tail: Scalar is idle, split the two PSUM
                        # evacuation copies across Scalar and DVE
                        nc.scalar.copy(ot[:, 0:W], pos[0][:])
                        nc.vector.tensor_copy(ot[:, W:2 * W], pos[1][:])
                    else:
                        for nn in range(2):
                            nc.vector.tensor_copy(
                                ot[:, nn * W:(nn + 1) * W], pos[nn][:])
                    if q is None:
                        q = [nc.sync, nc.gpsimd][tt % 2]
                    q.dma_start(out_d.ap()[tt * P:(tt + 1) * P, :], ot[:])

                # Global S-emission cursor kept ~14 chunks ahead of PV
                # consumption; pull_s first emits any q/k column blocks its
                # chunk depends on. Pass 0 S tiles were pre-built in stage B.
                sq = [(c2, m2, j2) for c2 in range(1, 4) for m2 in range(NP)
                      for j2 in range(4 * c2 + 4)]
                sq_i = 0
                pts_q = {(0, m2): list(pass0_pts[m2]) for m2 in range(NP)}
                state = {"ahead": sum(len(v) for v in pts_q.values())}

                def pull_s():
                    c2, m2, j2 = sq[sq_i]
                    ensure_qk(m2, c2)          # qT columns of this pass
                    for tqk in range(1, c2 + 1):
                        ensure_qk(m2, tqk)     # kT columns up to this pass
                    pt = emit_s(m2, j2, c2 * W, (c2 + 1) * W)
                    pts_q.setdefault((c2, m2), []).append((j2, pt))
                    state["ahead"] += 1

                # proj tiles of pass c-1 interleave with pass c, placed so
                # every yT write they read is already emitted
                proj_hooks = {}
                for c2 in range(1, 4):
                    proj_hooks[c2, 1] = [4 * (c2 - 1)]
                    proj_hooks[c2, 2] = [4 * (c2 - 1) + 1]
                    proj_hooks[c2, 3] = [4 * (c2 - 1) + 2, 4 * (c2 - 1) + 3]

                for c in range(4):
                    plo, phi = c * W, (c + 1) * W
                    jmax = 4 * c + 3
                    # v-tiles this pass reads must be emitted before its
                    # first PV; later fillers keep trickling
                    pump_v_upto(jmax)
                    for m in range(NP):
                        hA, hB = 2 * m, 2 * m + 1
                        ytA = psY.tile([MV, W], dt32, tag="yt", name="ytA")
                        ytB = psY.tile([MV, W], dt32, tag="yt", name="ytB")
                        while (len(pts_q.get((c, m), [])) < jmax + 1
                               and sq_i < len(sq)):
                            pull_s()
                            sq_i += 1
                            pump()
                        for pj, ppt in pts_q.pop((c, m)):
                            emit_pv(hA, pj, ppt, ytA, plo, phi, jmax)
                            emit_pv(hB, pj, ppt, ytB, plo, phi, jmax)
                            state["ahead"] -= 1
                            while state["ahead"] < 16 and sq_i < len(sq):
                                pull_s()
                                sq_i += 1
                                pump()
                        if c < 3 or m < NP - 1:
                            finish_front(hA, c, ytA, plo, phi)
                            finish_front(hB, c, ytB, plo, phi)
                            finish_step()
                            for tt in proj_hooks.get((c, m), []):
                                fillers.extend(
                                    ("p", -1, f)
                                    for f in make_proj_pieces(tt))
                        else:
                            finish_flush()
                            for tt in proj_hooks.get((c, m), []):
                                fillers.extend(
                                    ("p", -1, f)
                                    for f in make_proj_pieces(tt))
                            pump(len(fillers))
                            # tail: sq is exhausted here, psS slots are free
                            # for the last four proj tiles; copies split
                            # across Scalar+DVE, stores spread over queues
                            pop12 = psS.tile([P, 2, W], dt32, tag="s",
                                             name="pop12")
                            pos12 = [pop12[:, 0, :], pop12[:, 1, :]]
                            emit_proj_mms(12, pos12, 0, NP - 1)
                            finish_fast(hA, ytA, plo, phi)
                            finish_fast(hB, ytB, plo, phi)
                            emit_proj_mms(12, pos12, NP - 1, NP)
                            emit_proj_done(12, pos12, split=True,
                                           q=nc.sync)
                            pop13 = psS.tile([P, 2, W], dt32, tag="s",
                                             name="pop13")
                            pos13 = [pop13[:, 0, :], pop13[:, 1, :]]
                            emit_proj_mms(13, pos13, 0, NP)
                            emit_proj_done(13, pos13, split=True,
                                           q=nc.scalar)
                            pop14 = psS.tile([P, 2, W], dt32, tag="s",
                                             name="pop14")
                            pos14 = [pop14[:, 0, :], pop14[:, 1, :]]
                            emit_proj_mms(14, pos14, 0, NP)
                            emit_proj_done(14, pos14, split=True,
                                           q=nc.sync)
                            pop15 = psS.tile([P, 2, W], dt32, tag="s",
                                             name="pop15")
                            pos15 = [pop15[:, 0, :], pop15[:, 1, :]]
                            emit_proj_mms(15, pos15, 0, NP)
                            emit_proj_done(15, pos15, split=True,
                                           q=nc.scalar)
            outer.close()

    nc.compile()
    return nc


def make_core_inputs(x, W_attn, b_attn, W_proj, n_cores=8, HC=8, D=64):
    """Host-side sharding: per-core input dicts."""
    B, T, C = x.shape
    CO = HC * D
    NP = CO // P
    bf = _bf16_np()
    f8 = _f8_np()
    in_maps = []
    for c in range(n_cores):
        b = c // (n_cores // B)
        h0 = (c % (n_cores // B)) * HC
        lo = h0 * D
        bq = b_attn[lo:lo + CO]
        bv = b_attn[2 * C + lo:2 * C + lo + CO]
        xtb = x[b].T  # [C, T]
        T = xtb.shape[1]
        C_ = xtb.shape[0]
        KC = C_ // P
        KP = KC // 2

        def pmaj(a, *groups):
            # [C_, n] with C_=(g0 g1 .. p) -> [P, g0, g1, .., n]
            shp = list(groups) + [P, a.shape[1]]
            nd = len(shp)
            perm = [nd - 2] + list(range(nd - 2)) + [nd - 1]
            return np.ascontiguousarray(a.reshape(shp).transpose(perm))

        in_maps.append({
            "xt": pmaj(xtb.astype(bf), KC),
            "xt8": pmaj(xtb.astype(f8), KP, 2),
            "wq": pmaj((W_attn[:, lo:lo + CO] * WS).astype(f8), KP, 2),
            "wk": pmaj((W_attn[:, C + lo:C + lo + CO] * WS).astype(f8),
                       KP, 2),
            "wv": pmaj(W_attn[:, 2 * C + lo:2 * C + lo + CO].astype(bf),
                       KC),
            "bq": np.ascontiguousarray(bq.reshape(NP, P).T),
            "bvb": np.tile(bv[None, :], (P, 1)),
            "wp": pmaj(W_proj[lo:lo + CO, :].astype(bf), NP),
        })
    return in_maps


_CACHE = {}


def _get_program():
    if "nc" not in _CACHE:
        _CACHE["nc"] = build_program()
    return _CACHE["nc"]


def run_on_cores(x, W_attn, b_attn, W_proj, b_proj, trace=False):
    """Returns (full output [B,T,C], BassKernelResults)."""
    from concourse.bass_utils import run_bass_kernel_spmd

    x = np.asarray(x, np.float32)
    W_attn = np.asarray(W_attn, np.float32)
    b_attn = np.asarray(b_attn, np.float32)
    W_proj = np.asarray(W_proj, np.float32)
    b_proj = np.asarray(b_proj, np.float32)

    nc = _get_program()
    in_maps = make_core_inputs(x, W_attn, b_attn, W_proj)
    res = run_bass_kernel_spmd(nc, in_maps, core_ids=list(range(8)), trace=trace)
    B, T, C = x.shape
    out = np.empty((B, T, C), np.float32)
    for b in range(B):
        out[b] = (res.results[2 * b]["out"].astype(np.float32)
                  + res.results[2 * b + 1]["out"].astype(np.float32)
                  + b_proj[None, :])
    return out, res


def kernel(x, W_attn, b_attn, W_proj, b_proj):
    out, _ = run_on_cores(x, W_attn, b_attn, W_proj, b_proj, trace=False)
    return out


# revision 14
# speedup vs baseline: 1.3671x; 1.0102x over previous
"""Causal self-attention (B=4, T=2048, C=1024, H=16) on 8 trn2 NeuronCores.

Sharding: core c -> batch b = c//2, heads h0 = (c%2)*8 .. h0+8 (tensor
parallel over heads: c_attn columns / c_proj rows split). Each core computes a
partial projection output [T, C] in bf16; the host sums the two partials per
batch and adds b_proj.

Device-side dataflow (the kernel is exp-stream-bound: the Scalar engine's
~150us of exp is the critical path, so everything is organized around
starting it early and never starving it):
  - host passes x[b] pre-transposed twice: xt [C, T] bf16 (v path) and
    xt8 [C, T] fp8e4 (q/k path); wq/wk are fp8e4 scaled by 32
  - q/k projections: fp8 DoubleRow matmuls folding two 128-row contraction
    tiles per pass (2x PE throughput); PSUM evacuated on DVE tensor_scalar
    (scale 1/32 + bq bias for q; bk is DROPPED entirely - a per-query-constant
    logit shift cancels in softmax). Softmax tolerates the ~2% fp8 q/k noise;
    v and the output path stay bf16.
  - loads are 12 large DMAs on the sync/gpsimd queues only (descriptor
    issuance on the scalar queue would delay the first exp)
  - stage B proper is only the 0:512 column block of q/k per head pair plus
    its pass-0 S tiles and v tiles 0..3: first exp fires ~10us in. The
    remaining q/k column blocks and v tiles run as fillers inside the
    attention loop, emitted on demand (pull_s emits the q/k blocks a pass
    needs before its S tiles).
  - S^T tiles: TWO concurrent row-tiled bf16 matmuls (contraction 64 each,
    array rows 0:63 / 64:127) into one psS tile [128,2,512]
  - P~ = exp(S^T/8) on ScalarE, one instruction per psS tile (both heads);
    diagonal 128x128 blocks masked with an upper-triangular mask on DVE
  - attention runs in four 512-wide column passes; the output projection for
    pass c-1 overlaps the attention of pass c; the final four projection
    tiles are pipelined against the last pass's normalizes with copies split
    across Scalar and DVE
  - yT_aug [MV, 1024] += vaug_tile^T . P~ accumulated in PSUM over k-tiles
  - normalize: DVE reciprocal of denom row, gpsimd partition-broadcast,
    DVE multiply into yT (bf16)
"""

import numpy as np

P = 128


def _bf16_np():
    import ml_dtypes
    return ml_dtypes.bfloat16


def _f8_np():
    import ml_dtypes
    return ml_dtypes.float8_e4m3


WS = 32.0  # host-side scale on wq/wk before fp8 quantization


def build_program(T=2048, C=1024, HC=8, D=64, num_devices=8, trn="TRN2"):
    import concourse.mybir as mybir
    import concourse.tile as tile
    from concourse import bacc
    from concourse.masks import make_upper_triangular

    W = 512          # matmul moving-dim chunk (PSUM bank)
    KC = C // P      # contraction tiles over C (8)
    KP = KC // 2     # fp8 DoubleRow contraction pair-tiles (4)
    CO = HC * D      # this core's qkv channel block (512)
    NP = CO // P     # head pairs (4)
    TT = T // P      # k tiles (16)
    MV = 66          # PV stationary cols: 64 v-dims + ones + 1 pad
    dt32 = mybir.dt.float32
    bf16 = mybir.dt.bfloat16
    fp8 = mybir.dt.float8e4
    ActF = mybir.ActivationFunctionType
    Alu = mybir.AluOpType
    DR = mybir.MatmulPerfMode.DoubleRow
    scale = 1.0 / float(np.sqrt(D))

    nc = bacc.Bacc(trn, target_bir_lowering=False, debug=False,
                   enable_asserts=False, num_devices=num_devices)

    # all bulk inputs arrive pre-arranged partition-major on the host so
    # every load is one contiguous run per partition (descriptor generation
    # for strided gathers costs ~7us per DMA on the issuing queue)
    xt_d = nc.dram_tensor("xt", [P, KC, T], bf16, kind="ExternalInput")
    xt8_d = nc.dram_tensor("xt8", [P, KP, 2, T], fp8, kind="ExternalInput")
    wq_d = nc.dram_tensor("wq", [P, KP, 2, CO], fp8, kind="ExternalInput")
    wk_d = nc.dram_tensor("wk", [P, KP, 2, CO], fp8, kind="ExternalInput")
    wv_d = nc.dram_tensor("wv", [P, KC, CO], bf16, kind="ExternalInput")
    bq_d = nc.dram_tensor("bq", [P, NP], dt32, kind="ExternalInput")
    bvb_d = nc.dram_tensor("bvb", [P, CO], dt32, kind="ExternalInput")
    wp_d = nc.dram_tensor("wp", [P, NP, C], bf16, kind="ExternalInput")
    out_d = nc.dram_tensor("out", [T, C], bf16, kind="ExternalOutput")
    lsc_d = nc.dram_tensor("lsc", [HC, T], dt32)
    lsc2_d = nc.dram_tensor("lsc2", [HC, T], dt32)

    with tile.TileContext(nc) as tc:
        with tc.tile_pool(name="const", bufs=1) as cpool, \
             tc.tile_pool(name="pers", bufs=1) as pers:
            tri2 = cpool.tile([P, 2, P], bf16)
            make_upper_triangular(nc, tri2[:, 0, :], val=1.0, diag=True)
            make_upper_triangular(nc, tri2[:, 1, :], val=1.0, diag=True)
            bq_sb = cpool.tile([P, NP], dt32)
            bvb_sb = cpool.tile([P, CO], dt32)
            wpsb = cpool.tile([P, NP, C], bf16)

            qT = pers.tile([P, NP, T], bf16, tag="qT")
            kT = pers.tile([P, NP, T], bf16, tag="kT")
            vaug = pers.tile([P, TT, HC, MV], bf16, tag="vaug")
            yT = pers.tile([P, NP, T], bf16, tag="yT")
            nc.vector.memset(vaug[:, :, :, D:D + 1], 1.0)

            # big consolidated input tiles (one DMA each)
            xts = pers.tile([P, KC, T], bf16, tag="xts")
            x8t = pers.tile([P, KP, 2, T], fp8, tag="x8t")
            wqt = pers.tile([P, KP, 2, CO], fp8, tag="wqt")
            wkt = pers.tile([P, KP, 2, CO], fp8, tag="wkt")
            wvt = pers.tile([P, KC, CO], bf16, tag="wvt")

            from contextlib import ExitStack
            outer = ExitStack()
            ptpool = outer.enter_context(tc.tile_pool(name="ptp", bufs=20))
            psS = outer.enter_context(
                tc.tile_pool(name="psS", bufs=2, space="PSUM"))

            # ---- loads: sync/gpsimd only, critical first ------------------
            # first exp needs only x8 column block 0 + wq/wk head pair 0:
            # load those six small chunks first, everything else after
            di = [0]

            def dma(dst, src):
                [nc.sync, nc.gpsimd][di[0] % 2].dma_start(dst, src)
                di[0] += 1

            for kk in range(KP):
                dma(x8t[:, kk, :, 0:W], xt8_d.ap()[:, kk, :, 0:W])
            dma(wqt[:, :, :, 0:P], wq_d.ap()[:, :, :, 0:P])
            dma(wkt[:, :, :, 0:P], wk_d.ap()[:, :, :, 0:P])
            nc.sync.dma_start(bq_sb[:], bq_d.ap())
            for m in range(1, NP):
                dma(wqt[:, :, :, m * P:(m + 1) * P],
                    wq_d.ap()[:, :, :, m * P:(m + 1) * P])
                dma(wkt[:, :, :, m * P:(m + 1) * P],
                    wk_d.ap()[:, :, :, m * P:(m + 1) * P])
            # v tiles 0..3 (needed ~30us in, before pass-0 PV) read only
            # xt columns 0:512: column-split the xt load so they are not
            # gated on the full 4MB transfer
            dma(xts[:, :, 0:W], xt_d.ap()[:, :, 0:W])
            dma(wvt[:], wv_d.ap())
            nc.sync.dma_start(bvb_sb[:], bvb_d.ap())
            for kk in range(KP):
                dma(x8t[:, kk, :, W:2 * W], xt8_d.ap()[:, kk, :, W:2 * W])
            dma(xts[:, :, W:2 * W], xt_d.ap()[:, :, W:2 * W])
            for tq in range(2, NP):
                for kk in range(KP):
                    dma(x8t[:, kk, :, tq * W:(tq + 1) * W],
                        xt8_d.ap()[:, kk, :, tq * W:(tq + 1) * W])
            dma(xts[:, :, 2 * W:3 * W], xt_d.ap()[:, :, 2 * W:3 * W])
            dma(xts[:, :, 3 * W:4 * W], xt_d.ap()[:, :, 3 * W:4 * W])
            nc.gpsimd.dma_start(wpsb[:], wp_d.ap())
            bvb_v = bvb_sb[:].rearrange("p (h d) -> p h d", d=D)

            def emit_s(m, j, plo, phi):
                """Paired S^T + exp for heads (2m, 2m+1), k-tile j,
                columns [max(jb,plo), phi). Returns the pt pair tile."""
                jb = j * P
                qlo = max(jb, plo)
                w = phi - qlo
                pt = ptpool.tile([P, 2, W], bf16, tag="pt")
                sps = psS.tile([P, 2, W], dt32, tag="s")
                nc.tensor.matmul(
                    sps[:, 0, 0:w],
                    kT[0:D, m, jb:jb + P],
                    qT[0:D, m, qlo:phi],
                    start=True, stop=True, skip_group_check=True)
                nc.tensor.matmul(
                    sps[:, 1, 0:w],
                    kT[D:P, m, jb:jb + P],
                    qT[D:P, m, qlo:phi],
                    start=True, stop=True, skip_group_check=True)
                nc.scalar.activation(
                    pt[:, :, 0:w], sps[:, :, 0:w], ActF.Exp, scale=scale)
                if jb >= plo:  # diagonal block lives in this pass
                    nc.vector.tensor_mul(pt[:, :, 0:P], pt[:, :, 0:P],
                                         tri2[:])
                return pt

            def qk_group(wt, dst, m, tq, bias, pool):
                ps = pool.tile([P, W], dt32, tag=pool_tag[id(pool)],
                               name=f"qk_{m}_{tq}")
                for kk in range(KP):
                    nc.tensor.matmul(
                        ps[:],
                        wt[:, kk, :, m * P:(m + 1) * P],
                        x8t[:, kk, :, tq * W:(tq + 1) * W],
                        start=(kk == 0), stop=(kk == KP - 1),
                        perf_mode=DR, skip_group_check=True)
                if bias is not None:
                    nc.vector.tensor_scalar(
                        out=dst[:, m, tq * W:(tq + 1) * W], in0=ps[:],
                        scalar1=1.0 / WS, scalar2=bias,
                        op0=Alu.mult, op1=Alu.add)
                else:
                    nc.vector.tensor_scalar(
                        out=dst[:, m, tq * W:(tq + 1) * W], in0=ps[:],
                        scalar1=1.0 / WS, scalar2=None, op0=Alu.mult)

            def v_evac(tt, ps):
                nc.vector.scalar_tensor_tensor(
                    out=vaug[:, tt, :, 0:D],
                    in0=ps[:].rearrange("p (h d) -> p h d", d=D),
                    scalar=1.0, in1=bvb_v,
                    op0=Alu.mult, op1=Alu.add)

            # ---- stage B: q/k column block 0 + pass-0 S + v tiles 0..3 ---
            pass0_pts = [[] for _ in range(NP)]
            pool_tag = {}
            with nc.named_scope("qkv"), \
                 tc.tile_pool(name="psB", bufs=4, space="PSUM") as psB:
                pool_tag[id(psB)] = "psB"
                for m in range(NP):
                    qk_group(wqt, qT, m, 0, bq_sb[:, m:m + 1], psB)
                    qk_group(wkt, kT, m, 0, None, psB)
                    for j in range(4):
                        pass0_pts[m].append((j, emit_s(m, j, 0, W)))
                    if m >= 2:  # v tiles trickle in once xt cols 0:512 land
                        ps = psB.tile([P, CO], dt32, tag="psB", name="vps")
                        for kc in range(KC):
                            nc.tensor.matmul(
                                ps[:],
                                xts[:, kc, (m - 2) * P:(m - 1) * P],
                                wvt[:, kc, :],
                                start=(kc == 0), stop=(kc == KC - 1))
                        v_evac(m - 2, ps)
                for tt in range(2, 4):
                    ps = psB.tile([P, CO], dt32, tag="psB", name="vps")
                    for kc in range(KC):
                        nc.tensor.matmul(
                            ps[:],
                            xts[:, kc, tt * P:(tt + 1) * P],
                            wvt[:, kc, :],
                            start=(kc == 0), stop=(kc == KC - 1))
                    v_evac(tt, ps)

            # ------- attention + projection -------------------------------
            # four 512-wide column sub-passes; pass c consumes k-tiles
            # j <= 4c+3. q/k column blocks 1..3, v tiles 4..15 and the
            # previous pass's projection tiles all run as fillers trickled
            # between S chunks so no engine bursts ever starve the exp
            # stream.
            with nc.named_scope("attn"), \
                 tc.tile_pool(name="nrm", bufs=4) as nrmpool, \
                 tc.tile_pool(name="ost", bufs=2) as opool, \
                 tc.tile_pool(name="psY", bufs=3, space="PSUM") as psY, \
                 tc.tile_pool(name="psO", bufs=1, space="PSUM") as psO:
                pool_tag[id(psO)] = "o"

                def make_qk_piece(m, tq):
                    def q_piece():
                        qk_group(wqt, qT, m, tq, bq_sb[:, m:m + 1], psO)

                    def k_piece():
                        qk_group(wkt, kT, m, tq, None, psO)

                    return q_piece, k_piece

                def make_v_pieces(tt):
                    st = {}

                    def p1():
                        st["ps"] = psO.tile([P, CO], dt32, tag="o",
                                            name="vps")
                        for kc in range(KC // 2):
                            nc.tensor.matmul(
                                st["ps"][:],
                                xts[:, kc, tt * P:(tt + 1) * P],
                                wvt[:, kc, :],
                                start=(kc == 0), stop=False)

                    def p2():
                        for kc in range(KC // 2, KC):
                            nc.tensor.matmul(
                                st["ps"][:],
                                xts[:, kc, tt * P:(tt + 1) * P],
                                wvt[:, kc, :],
                                start=False, stop=(kc == KC - 1))
                        v_evac(tt, st["ps"])

                    return [p1, p2]

                def make_proj_pieces(tt):
                    st = {}

                    def mk(nn, half):
                        def piece():
                            if half == 0:
                                st[nn] = psO.tile([P, W], dt32, tag="o",
                                                  name=f"po{nn}")
                                if nn == 0:
                                    st["ot"] = opool.tile([P, C], bf16,
                                                          tag="ot",
                                                          name="ot")
                                kts = (0, 1)
                            else:
                                kts = (2, 3)
                            for kt in kts:
                                nc.tensor.matmul(
                                    st[nn][:],
                                    yT[:, kt, tt * P:(tt + 1) * P],
                                    wpsb[:, kt, nn * W:(nn + 1) * W],
                                    start=(kt == 0), stop=(kt == NP - 1),
                                    skip_group_check=True)
                            if half == 1:
                                nc.vector.tensor_copy(
                                    st["ot"][:, nn * W:(nn + 1) * W],
                                    st[nn][:])
                                if nn == 1:
                                    [nc.sync, nc.gpsimd][tt % 2].dma_start(
                                        out_d.ap()[tt * P:(tt + 1) * P, :],
                                        st["ot"][:])
                        return piece

                    return [mk(0, 0), mk(0, 1), mk(1, 0), mk(1, 1)]

                # filler entries: (kind, key, fn); kind 'v' keyed by tt,
                # kind 'qk' keyed by (m, tq), kind 'p' for proj pieces
                fillers = []
                qk_pending = {}
                for tq in range(1, NP):
                    for m in range(NP):
                        qp, kp = make_qk_piece(m, tq)
                        fillers.append(("qk", (m, tq, "q"), qp))
                        fillers.append(("qk", (m, tq, "k"), kp))
                        qk_pending[(m, tq, "q")] = qp
                        qk_pending[(m, tq, "k")] = kp
                    for tt in range(4 * tq, 4 * tq + 4):
                        fillers.extend(
                            ("v", tt, f) for f in make_v_pieces(tt))

                def run_entry(e):
                    """Returns True if real work was emitted."""
                    kind, key, fn = e
                    if kind == "qk":
                        if key in qk_pending:
                            del qk_pending[key]
                            fn()
                            return True
                        return False
                    fn()
                    return True

                def pump(n=1):
                    done = 0
                    while done < n and fillers:
                        if run_entry(fillers.pop(0)):
                            done += 1

                def ensure_qk(m, tq):
                    for kq in ("q", "k"):
                        key = (m, tq, kq)
                        if key in qk_pending:
                            fn = qk_pending.pop(key)
                            fn()

                def pump_v_upto(tt):
                    while fillers and any(
                            k == "v" and key <= tt
                            for k, key, _ in fillers):
                        run_entry(fillers.pop(0))

                def emit_pv(h, j, pt, yt, plo, phi, jmax):
                    jb = j * P
                    qlo = max(jb, plo)
                    nc.tensor.matmul(
                        yt[:, qlo - plo:phi - plo],
                        vaug[:, j, h, :],
                        pt[:, h % 2, 0:phi - qlo],
                        start=(j == 0), stop=(j == jmax),
                        skip_group_check=True)

                # finish is a 3-stage pipeline across head-pairs so no DVE op
                # ever waits at the head of the queue on an in-flight DMA:
                #   front: evacuate yt PSUM + kick the denom-row fold DMAs
                #   mid (a pair later): reciprocal + kick the broadcast DMAs
                #   back (another pair later): normalize-multiply into yT
                fin_q1, fin_q2 = [], []

                def finish_front(h, c, yt, plo, phi):
                    ys = nrmpool.tile([D + 1, W], dt32, tag="ys")
                    nc.vector.tensor_copy(ys[:], yt[0:D + 1, :])
                    nc.sync.dma_start(
                        lsc_d.ap()[h, plo:phi].rearrange("(o t) -> o t", o=1),
                        ys[D:D + 1, :])
                    dn = nrmpool.tile([P, W // P], dt32, tag="dn")
                    nc.gpsimd.dma_start(
                        dn[:],
                        lsc_d.ap()[h, plo:phi].rearrange("(p c) -> p c", p=P))
                    fin_q1.append((h, ys, dn, plo, phi))

                def finish_mid(st):
                    h, ys, dn, plo, phi = st
                    nc.vector.reciprocal(dn[:], dn[:])
                    nc.gpsimd.dma_start(
                        lsc2_d.ap()[h, plo:phi].rearrange("(p c) -> p c", p=P),
                        dn[:])
                    bc = nrmpool.tile([D, W], dt32, tag="bc")
                    nc.sync.dma_start(
                        bc[:],
                        lsc2_d.ap()[h, plo:phi].rearrange(
                            "(o t) -> o t", o=1).broadcast_to([D, W]))
                    fin_q2.append((h, ys, bc, plo, phi))

                def finish_back(st):
                    h, ys, bc, plo, phi = st
                    r0 = (h % 2) * D
                    nc.vector.tensor_mul(
                        yT[r0:r0 + D, h // 2, plo:phi], ys[0:D, :], bc[:])

                def finish_step():
                    while len(fin_q1) > 2:
                        finish_mid(fin_q1.pop(0))
                    while len(fin_q2) > 2:
                        finish_back(fin_q2.pop(0))

                def finish_flush():
                    while fin_q1:
                        finish_mid(fin_q1.pop(0))
                    while fin_q2:
                        finish_back(fin_q2.pop(0))

                def finish_fast(h, yt, plo, phi):
                    """DMA-free normalize (gpsimd broadcast + fast DVE
                    reciprocal) — low latency, for the last column pass."""
                    drow = nrmpool.tile([1, W], dt32, tag="drow")
                    nc.vector.tensor_copy(drow[:], yt[D:D + 1, :])
                    ys = nrmpool.tile([D + 1, W], dt32, tag="ys")
                    nc.vector.tensor_copy(ys[0:D, :], yt[0:D, :])
                    bc = nrmpool.tile([D, W], dt32, tag="bc")
                    nc.gpsimd.partition_broadcast(bc[:], drow[:], channels=D)
                    rec = nrmpool.tile([D, W], dt32, tag="bc", name="rec")
                    nc.vector.reciprocal_approx_fast(out=rec[:], in_=bc[:])
                    r0 = (h % 2) * D
                    nc.vector.tensor_mul(
                        yT[r0:r0 + D, h // 2, plo:phi], ys[0:D, :], rec[:])

                def emit_proj_mms(tt, pos, k0, k1):
                    for kt in range(k0, k1):
                        for nn in range(2):
                            nc.tensor.matmul(
                                pos[nn][:],
                                yT[:, kt, tt * P:(tt + 1) * P],
                                wpsb[:, kt, nn * W:(nn + 1) * W],
                                start=(kt == 0), stop=(kt == NP - 1),
                                skip_group_check=True)

                def emit_proj_done(tt, pos, split=False, q=None):
                    ot = opool.tile([P, C], bf16, tag="ot")
                    if split:
                        # kernel tail: Scalar is idle, split the two PSUM
                        # evacuation copies across Scalar and DVE
                        nc.scalar.copy(ot[:, 0:W], pos[0][:])
                        nc.vector.tensor_copy(ot[:, W:2 * W], pos[1][:])
                    else:
                        for nn in range(2):
                            nc.vector.tensor_copy(
                                ot[:, nn * W:(nn + 1) * W], pos[nn][:])
                    if q is None:
                        q = [nc.sync, nc.gpsimd][tt % 2]
                    q.dma_start(out_d.ap()[tt * P:(tt + 1) * P, :], ot[:])

                # Global S-emission cursor kept ~14 chunks ahead of PV
                # consumption; pull_s first emits any q/k column blocks its
                # chunk depends on. Pass 0 S tiles were pre-built in stage B.
                sq = [(c2, m2, j2) for c2 in range(1, 4) for m2 in range(NP)
                      for j2 in range(4 * c2 + 4)]
                sq_i = 0
                pts_q = {(0, m2): list(pass0_pts[m2]) for m2 in range(NP)}
                state = {"ahead": sum(len(v) for v in pts_q.values())}

                def pull_s():
                    c2, m2, j2 = sq[sq_i]
                    ensure_qk(m2, c2)          # qT columns of this pass
                    for tqk in range(1, c2 + 1):
                        ensure_qk(m2, tqk)     # kT columns up to this pass
                    pt = emit_s(m2, j2, c2 * W, (c2 + 1) * W)
                    pts_q.setdefault((c2, m2), []).append((j2, pt))
                    state["ahead"] += 1

                # proj tiles of pass c-1 interleave with pass c, placed so
                # every yT write they read is already emitted
                proj_hooks = {}
                for c2 in range(1, 4):
                    proj_hooks[c2, 1] = [4 * (c2 - 1)]
                    proj_hooks[c2, 2] = [4 * (c2 - 1) + 1]
                    proj_hooks[c2, 3] = [4 * (c2 - 1) + 2, 4 * (c2 - 1) + 3]

                for c in range(4):
                    plo, phi = c * W, (c + 1) * W
                    jmax = 4 * c + 3
                    # v-tiles this pass reads must be emitted before its
                    # first PV; later fillers keep trickling
                    pump_v_upto(jmax)
                    for m in range(NP):
                        hA, hB = 2 * m, 2 * m + 1
                        ytA = psY.tile([MV, W], dt32, tag="yt", name="ytA")
                        ytB = psY.tile([MV, W], dt32, tag="yt", name="ytB")
                        while (len(pts_q.get((c, m), [])) < jmax + 1
                               and sq_i < len(sq)):
                            pull_s()
                            sq_i += 1
                            pump()
                        for pj, ppt in pts_q.pop((c, m)):
                            emit_pv(hA, pj, ppt, ytA, plo, phi, jmax)
                            emit_pv(hB, pj, ppt, ytB, plo, phi, jmax)
                            state["ahead"] -= 1
                            while state["ahead"] < 16 and sq_i < len(sq):
                                pull_s()
                                sq_i += 1
                                pump()
                        if c < 3 or m < NP - 1:
                            finish_front(hA, c, ytA, plo, phi)
                            finish_front(hB, c, ytB, plo, phi)
                            finish_step()
                            for tt in proj_hooks.get((c, m), []):
                                fillers.extend(
                                    ("p", -1, f)
                                    for f in make_proj_pieces(tt))
                        else:
                            finish_flush()
                            for tt in proj_hooks.get((c, m), []):
                                fillers.extend(
                                    ("p", -1, f)
                                    for f in make_proj_pieces(tt))
                            pump(len(fillers))
                            # tail: sq is exhausted here, psS slots are free
                            # for the last four proj tiles; copies split
                            # across Scalar+DVE, stores spread over queues
                            pop12 = psS.tile([P, 2, W], dt32, tag="s",
                                             name="pop12")
                            pos12 = [pop12[:, 0, :], pop12[:, 1, :]]
                            emit_proj_mms(12, pos12, 0, NP - 1)
                            finish_fast(hA, ytA, plo, phi)
                            finish_fast(hB, ytB, plo, phi)
                            emit_proj_mms(12, pos12, NP - 1, NP)
                            emit_proj_done(12, pos12, split=True,
                                           q=nc.sync)
                            pop13 = psS.tile([P, 2, W], dt32, tag="s",
                                             name="pop13")
                            pos13 = [pop13[:, 0, :], pop13[:, 1, :]]
                            emit_proj_mms(13, pos13, 0, NP)
                            emit_proj_done(13, pos13, split=True,
                                           q=nc.scalar)
                            pop14 = psS.tile([P, 2, W], dt32, tag="s",
                                             name="pop14")
                            pos14 = [pop14[:, 0, :], pop14[:, 1, :]]
                            emit_proj_mms(14, pos14, 0, NP)
                            emit_proj_done(14, pos14, split=True,
                                           q=nc.sync)
                            pop15 = psS.tile([P, 2, W], dt32, tag="s",
                                             name="pop15")
                            pos15 = [pop15[:, 0, :], pop15[:, 1, :]]
                            emit_proj_mms(15, pos15, 0, NP)
                            emit_proj_done(15, pos15, split=True,
                                           q=nc.scalar)
            outer.close()

    nc.compile()
    return nc


def make_core_inputs(x, W_attn, b_attn, W_proj, n_cores=8, HC=8, D=64):
    """Host-side sharding: per-core input dicts."""
    B, T, C = x.shape
    CO = HC * D
    NP = CO // P
    bf = _bf16_np()
    f8 = _f8_np()
    in_maps = []
    for c in range(n_cores):
        b = c // (n_cores // B)
        h0 = (c % (n_cores // B)) * HC
        lo = h0 * D
        bq = b_attn[lo:lo + CO]
        bv = b_attn[2 * C + lo:2 * C + lo + CO]
        xtb = x[b].T  # [C, T]
        T = xtb.shape[1]
        C_ = xtb.shape[0]
        KC = C_ // P
        KP = KC // 2

        def pmaj(a, *groups):
            # [C_, n] with C_=(g0 g1 .. p) -> [P, g0, g1, .., n]
            shp = list(groups) + [P, a.shape[1]]
            nd = len(shp)
            perm = [nd - 2] + list(range(nd - 2)) + [nd - 1]
            return np.ascontiguousarray(a.reshape(shp).transpose(perm))

        in_maps.append({
            "xt": pmaj(xtb.astype(bf), KC),
            "xt8": pmaj(xtb.astype(f8), KP, 2),
            "wq": pmaj((W_attn[:, lo:lo + CO] * WS).astype(f8), KP, 2),
            "wk": pmaj((W_attn[:, C + lo:C + lo + CO] * WS).astype(f8),
                       KP, 2),
            "wv": pmaj(W_attn[:, 2 * C + lo:2 * C + lo + CO].astype(bf),
                       KC),
            "bq": np.ascontiguousarray(bq.reshape(NP, P).T),
            "bvb": np.tile(bv[None, :], (P, 1)),
            "wp": pmaj(W_proj[lo:lo + CO, :].astype(bf), NP),
        })
    return in_maps


_CACHE = {}


def _get_program():
    if "nc" not in _CACHE:
        _CACHE["nc"] = build_program()
    return _CACHE["nc"]


def run_on_cores(x, W_attn, b_attn, W_proj, b_proj, trace=False):
    """Returns (full output [B,T,C], BassKernelResults)."""
    from concourse.bass_utils import run_bass_kernel_spmd

    x = np.asarray(x, np.float32)
    W_attn = np.asarray(W_attn, np.float32)
    b_attn = np.asarray(b_attn, np.float32)
    W_proj = np.asarray(W_proj, np.float32)
    b_proj = np.asarray(b_proj, np.float32)

    nc = _get_program()
    in_maps = make_core_inputs(x, W_attn, b_attn, W_proj)
    res = run_bass_kernel_spmd(nc, in_maps, core_ids=list(range(8)), trace=trace)
    B, T, C = x.shape
    out = np.empty((B, T, C), np.float32)
    for b in range(B):
        out[b] = (res.results[2 * b]["out"].astype(np.float32)
                  + res.results[2 * b + 1]["out"].astype(np.float32)
                  + b_proj[None, :])
    return out, res


def kernel(x, W_attn, b_attn, W_proj, b_proj):
    out, _ = run_on_cores(x, W_attn, b_attn, W_proj, b_proj, trace=False)
    return out
